# revision 4
# baseline (speedup 1.0000x reference)
"""CrossAttention kernel for Trainium2, data-parallel over batch on 8 cores.

Per example (paper P, review R are (dim, seq) f32):
    A = relu(W @ R + b)                               (o, i)   f32r
    B = relu(W @ P + b)                               (o, j)   f32r
    affT[j, i] = sum_o B[o, j] A[o, i]                j-tiles via PE
    C^T[j, i] = softmax_i(affT[j, :] / 32)            stored into B's dead
                                                      jj column-blocks
    s[i] = sum_j C^T[j, i]                            N=2 ones-matmul
    Pr[i, d] = sum_j C^T[j, i] P^T[j, d]              P^T via PE transposes
    r = R^T (exact, f32 PE transposes), Rp = r * s
    Rc = concat([r, Rp, Pr])

HW-validated numerics: f32r matmul rel err ~1.5e-4 (K=1024); f32r is IEEE
f32 with rounded mantissa, so f32r tiles bitcast to f32 transpose exactly;
ACT/DVE write f32r directly. PSUM: single 8-slot 1-bank pool everywhere.
"""

import numpy as np

import concourse.bacc as bacc
import concourse.mybir as mybir
import concourse.tile as tile
from concourse.bass_utils import run_bass_kernel_spmd

N_CORES = 8
BSZ, DIM, SEQ = 32, 1024, 1024
BPC = BSZ // N_CORES
NT = DIM // 128  # 8
FP32 = mybir.dt.float32
FP32R = mybir.dt.float32r
Relu = mybir.ActivationFunctionType.Relu
Exp = mybir.ActivationFunctionType.Exp


def build_nc():
    nc = bacc.Bacc()
    paper = nc.dram_tensor("paper", [BPC, DIM, SEQ], FP32, kind="ExternalInput")
    review = nc.dram_tensor("review", [BPC, DIM, SEQ], FP32, kind="ExternalInput")
    W = nc.dram_tensor("W", [DIM, DIM], FP32, kind="ExternalInput")
    bias = nc.dram_tensor("b", [DIM], FP32, kind="ExternalInput")
    Rp = nc.dram_tensor("Rp", [BPC, SEQ, DIM], FP32, kind="ExternalOutput")
    Pr = nc.dram_tensor("Pr", [BPC, SEQ, DIM], FP32, kind="ExternalOutput")
    Rc = nc.dram_tensor("Rc", [BPC, 3 * SEQ, DIM], FP32, kind="ExternalOutput")

    ident_dram = nc.inline_tensor(np.eye(128, dtype=np.float32), "ident_c")
    ones_dram = nc.inline_tensor(np.ones((128, 2), dtype=np.float32), "ones_c")

    with tile.TileContext(nc) as tc:
        with (
            tc.tile_pool(name="cst", bufs=1) as cst,
            tc.tile_pool(name="bigs", bufs=1) as bigs,
            tc.tile_pool(name="stream", bufs=5) as stream,
            tc.tile_pool(name="strf32", bufs=2) as strf32,
            tc.tile_pool(name="esb", bufs=1) as esb,
            tc.tile_pool(name="sbout", bufs=2) as sbout,
            tc.tile_pool(name="sml", bufs=4) as sml,
            tc.tile_pool(name="ps", bufs=8, space="PSUM") as ps,
        ):
            ident = cst.tile([128, 128], FP32, tag="ident")
            nc.sync.dma_start(out=ident, in_=ident_dram[:, :])
            ones2 = cst.tile([128, 2], FP32R, tag="ones2")
            nc.gpsimd.dma_start(out=ones2, in_=ones_dram[:, :])
            b_sb = cst.tile([128, NT], FP32, tag="b_sb")
            nc.sync.dma_start(out=b_sb, in_=bias.rearrange("(m p) -> p m", p=128))

            # ---- one-time: Wt[p, k, o] = W[o, k*128+p], f32r ----
            Wt = cst.tile([128, NT, DIM], FP32R, tag="Wt")
            for ko in range(NT):
                wk = stream.tile([128, 2, 512], FP32R, tag="rk2", name=f"wk{ko}")
                nc.gpsimd.dma_start(
                    out=wk,
                    in_=W[ko * 128:(ko + 1) * 128, :].rearrange(
                        "p (t s) -> p t s", t=2
                    ),
                )
                for g in range(2):
                    pst = ps.tile([128, 512], FP32, tag="mm", name=f"wt{ko}_{g}")
                    for q in range(4):
                        k = g * 4 + q
                        nc.tensor.transpose(
                            pst[:, q * 128:(q + 1) * 128],
                            wk[:, k // 4, (k % 4) * 128:(k % 4 + 1) * 128].bitcast(
                                FP32
                            ),
                            ident,
                        )
                    nc.scalar.copy(
                        Wt[:, g * 4:(g + 1) * 4, ko * 128:(ko + 1) * 128],
                        pst.rearrange("p (q n) -> p q n", q=4),
                    )

            A = bigs.tile([128, NT, SEQ], FP32R, tag="A")
            B = bigs.tile([128, NT, SEQ], FP32R, tag="B")
            PT = bigs.tile([128, NT, DIM], FP32R, tag="PT")

            def linear_half(src_dram, e, c, out_tile, transpose_to=None):
                """out_tile[:, :, c-half] = relu(W @ src + b); optionally also
                PE-transpose the streamed blocks into transpose_to (P^T)."""
                psums = [
                    ps.tile([128, 512], FP32, tag="mm", name=f"mm{e}_{c}_{m}")
                    for m in range(NT)
                ]
                rks = []
                for k2 in range(4):
                    rk = stream.tile(
                        [128, 2, 512], FP32R, tag="rk2", name=f"rk{e}_{c}_{k2}"
                    )
                    nc.gpsimd.dma_start(
                        out=rk,
                        in_=src_dram[
                            e, k2 * 256:(k2 + 1) * 256, c * 512:(c + 1) * 512
                        ].rearrange("(t p) s -> p t s", p=128),
                    )
                    rks.append(rk)
                    for t in range(2):
                        k = k2 * 2 + t
                        for m in range(NT):
                            nc.tensor.matmul(
                                psums[m],
                                lhsT=Wt[:, k, m * 128:(m + 1) * 128],
                                rhs=rk[:, t, :],
                                start=(k == 0),
                                stop=(k == NT - 1),
                            )
                for m in range(NT):
                    nc.scalar.activation(
                        out_tile[:, m, c * 512:(c + 1) * 512],
                        psums[m], Relu, bias=b_sb[:, m:m + 1],
                    )
                if transpose_to is not None:
                    # blocks (k, jj = c*4+q) -> PT[:, jj, k*128:...]
                    for k2 in range(4):
                        for t in range(2):
                            k = k2 * 2 + t
                            pst = ps.tile(
                                [128, 512], FP32, tag="mm", name=f"pt{e}_{c}_{k}"
                            )
                            for q in range(4):
                                nc.tensor.transpose(
                                    pst[:, q * 128:(q + 1) * 128],
                                    rks[k2][:, t, q * 128:(q + 1) * 128].bitcast(
                                        FP32
                                    ),
                                    ident,
                                )
                            nc.scalar.copy(
                                transpose_to.rearrange(
                                    "p j (k n) -> p j k n", n=128
                                )[:, c * 4:(c + 1) * 4, k, :],
                                pst.rearrange("p (q n) -> p q n", q=4),
                            )

            for e in range(BPC):
                for c in range(2):
                    linear_half(review, e, c, A)
                for c in range(2):
                    linear_half(paper, e, c, B, transpose_to=PT)

                # ---- phase 3: affinity^T + softmax -> C^T into B's jj blocks ----
                for jj in range(NT):
                    pa = ps.tile([128, 512], FP32, tag="mm", name=f"afa{e}_{jj}")
                    pb = ps.tile([128, 512], FP32, tag="mm", name=f"afb{e}_{jj}")
                    for c, psum in ((0, pa), (1, pb)):
                        for m in range(NT):
                            nc.tensor.matmul(
                                psum,
                                lhsT=B[:, m, jj * 128:(jj + 1) * 128],
                                rhs=A[:, m, c * 512:(c + 1) * 512],
                                start=(m == 0),
                                stop=(m == NT - 1),
                            )
                    nm_a = sml.tile([128, 1], FP32, tag="nm_a")
                    nm_b = sml.tile([128, 1], FP32, tag="nm_b")
                    nc.vector.tensor_reduce(
                        nm_a, pa, axis=mybir.AxisListType.X,
                        op=mybir.AluOpType.max, negate=True,
                    )
                    nc.vector.tensor_reduce(
                        nm_b, pb, axis=mybir.AxisListType.X,
                        op=mybir.AluOpType.max, negate=True,
                    )
                    negmax_s = sml.tile([128, 1], FP32, tag="negmax_s")
                    nc.vector.tensor_tensor(
                        negmax_s, nm_a, nm_b, op=mybir.AluOpType.min
                    )
                    nc.vector.tensor_scalar_mul(negmax_s, negmax_s, 1.0 / 32.0)
                    e_sb = esb.tile([128, SEQ], FP32, tag="e_sb")
                    d_a = sml.tile([128, 1], FP32, tag="d_a")
                    d_b = sml.tile([128, 1], FP32, tag="d_b")
                    nc.scalar.activation(
                        e_sb[:, 0:512], pa, Exp,
                        bias=negmax_s, scale=1.0 / 32.0, accum_out=d_a,
                    )
                    nc.scalar.activation(
                        e_sb[:, 512:1024], pb, Exp,
                        bias=negmax_s, scale=1.0 / 32.0, accum_out=d_b,
                    )
                    recip = sml.tile([128, 1], FP32, tag="recip")
                    nc.vector.tensor_tensor(
                        recip, d_a, d_b, op=mybir.AluOpType.add
                    )
                    nc.vector.reciprocal(recip, recip)
                    # C^T[jj*128+p, m*128+t] -> B[p, m, jj*128+t]
                    nc.vector.tensor_scalar_mul(
                        B[:, :, jj * 128:(jj + 1) * 128],
                        e_sb.rearrange("p (m t) -> p m t", t=128),
                        recip,
                    )

                # ---- phase 4: outputs per i-tile ----
                for ii in range(NT):
                    i0 = ii * 128
                    rcol = strf32.tile(
                        [128, NT, 128], FP32, tag="rcol", name=f"rcol{e}_{ii}"
                    )
                    nc.sync.dma_start(
                        out=rcol,
                        in_=review[e, :, i0:i0 + 128].rearrange(
                            "(k p) s -> p k s", p=128
                        ),
                    )
                    pr_a = ps.tile([128, 512], FP32, tag="mm", name=f"pra{e}_{ii}")
                    pr_b = ps.tile([128, 512], FP32, tag="mm", name=f"prb{e}_{ii}")
                    psum_s = ps.tile([128, 2], FP32, tag="mm", name=f"ps_s{e}_{ii}")
                    for jj in range(NT):
                        lhs = B[:, ii, jj * 128:(jj + 1) * 128]  # C^T slice
                        for c, psum in ((0, pr_a), (1, pr_b)):
                            nc.tensor.matmul(
                                psum,
                                lhsT=lhs,
                                rhs=PT[:, jj, c * 512:(c + 1) * 512],
                                start=(jj == 0),
                                stop=(jj == NT - 1),
                            )
                        nc.tensor.matmul(
                            psum_s, lhsT=lhs, rhs=ones2,
                            start=(jj == 0), stop=(jj == NT - 1),
                        )
                    r_sb = sbout.tile([128, SEQ], FP32, tag="r_sb")
                    for g in range(2):
                        pst = ps.tile(
                            [128, 512], FP32, tag="mm", name=f"rt{e}_{ii}_{g}"
                        )
                        for q in range(4):
                            k = g * 4 + q
                            nc.tensor.transpose(
                                pst[:, q * 128:(q + 1) * 128],
                                rcol[:, k, :],
                                ident,
                            )
                        nc.vector.tensor_copy(
                            r_sb[:, g * 512:(g + 1) * 512], pst
                        )
                    s_col = sml.tile([128, 1], FP32, tag="s_col")
                    nc.vector.tensor_copy(s_col, psum_s[:, 0:1])
                    rp_sb = sbout.tile([128, SEQ], FP32, tag="rp_sb")
                    nc.vector.tensor_scalar_mul(rp_sb, r_sb, s_col)
                    pr_sb = sbout.tile([128, SEQ], FP32, tag="pr_sb")
                    nc.vector.tensor_copy(pr_sb[:, 0:512], pr_a)
                    nc.vector.tensor_copy(pr_sb[:, 512:1024], pr_b)

                    nc.sync.dma_start(out=Rc[e, i0:i0 + 128, :], in_=r_sb)
                    nc.sync.dma_start(out=Rp[e, i0:i0 + 128, :], in_=rp_sb)
                    nc.sync.dma_start(
                        out=Rc[e, SEQ + i0:SEQ + i0 + 128, :], in_=rp_sb
                    )
                    nc.sync.dma_start(out=Pr[e, i0:i0 + 128, :], in_=pr_sb)
                    nc.sync.dma_start(
                        out=Rc[e, 2 * SEQ + i0:2 * SEQ + i0 + 128, :], in_=pr_sb
                    )
    nc.compile()
    return nc


_NC_CACHE = None


def _get_nc():
    global _NC_CACHE
    if _NC_CACHE is None:
        _NC_CACHE = build_nc()
    return _NC_CACHE


def kernel(paper, review, W, b, _trace=False):
    paper = np.ascontiguousarray(paper, dtype=np.float32)
    review = np.ascontiguousarray(review, dtype=np.float32)
    W = np.ascontiguousarray(W, dtype=np.float32)
    b = np.ascontiguousarray(b, dtype=np.float32)
    nc = _get_nc()
    in_maps = []
    for c in range(N_CORES):
        sl = slice(c * BPC, (c + 1) * BPC)
        in_maps.append(
            {"paper": paper[sl], "review": review[sl], "W": W, "b": b}
        )
    res = run_bass_kernel_spmd(
        nc, in_maps, core_ids=list(range(N_CORES)), trace=_trace
    )
    Rp_o = np.concatenate([res.results[c]["Rp"] for c in range(N_CORES)], axis=0)
    Pr_o = np.concatenate([res.results[c]["Pr"] for c in range(N_CORES)], axis=0)
    Rc_o = np.concatenate([res.results[c]["Rc"] for c in range(N_CORES)], axis=0)
    if _trace:
        kernel.last_exec_time_ns = res.exec_time_ns
        kernel.last_trace = res.instructions_and_trace
    return (Rp_o, Pr_o, Rc_o)
